# revision 3
# baseline (speedup 1.0000x reference)
"""Multi-head attention (N=4096, D=1024, 16 heads) on 8 trn2 NeuronCores.

Sharding: tensor-parallel over heads. Each core owns 2 heads (128 of the
1024 QKV projection columns / W_o rows), computes its heads' attention
fully on-device, applies its W_o row-slice, and returns a partial
[4096, 1024] output. The host sums the 8 partials (the "all-reduce").

Device kernel per core (all matmuls bf16, accumulation fp32 in PSUM):
  1. QT/KT = (W_qc^T q^T), stored [128=2*64 head dims, 4096] in SBUF.
     V stored row-major [row-tile 128, d 0:64 | ones | d 64:128 | ones].
  2. Per head h, per 1024-row q chunk, streaming over 32 key tiles:
     scores^T tile = K_h^T-slice @ Q_h-chunk -> PSUM [128, 1024];
     P = exp(scores/8) via ScalarE direct PSUM->SBUF bf16;
     [u; r]^T += (V_h | 1)^T P^T accumulated in PSUM [65, 1024]
     (row 64 is the softmax denominator r).
  3. normalize: recip(r) on DVE, broadcast across 64 partitions with a
     K=1 matmul against ones, u_norm = u * bcast -> bf16.
  4. out-partial = u_norm^T @ W_o-slice (two K=64 matmuls accumulated),
     DMA PSUM -> DRAM fp32.
"""

import numpy as np
import ml_dtypes

import concourse.bass as bass
import concourse.tile as tile
from concourse import bacc, mybir
from concourse.bass_utils import run_bass_kernel_spmd

BF16 = mybir.dt.bfloat16
F32 = mybir.dt.float32
EXP = mybir.ActivationFunctionType.Exp

N = 4096
DIN = 1024
DOUT = 1024
NCORES = 8
DPC = 128  # dims per core = 2 heads * 64
HD = 64


def emit(tc, outs, ins, n, din):
    nc = tc.nc
    qT, kT, vT, wq, wk, wv, wo = ins
    out = outs[0]

    nkt = din // 128          # contraction tiles for projections
    nch = n // 512            # 512-wide column chunks of QT/KT
    njt = n // 128            # key row tiles
    is_chunk = min(1024, n)   # q rows processed per attention sweep
    nis = n // is_chunk
    n_half = is_chunk // 512
    vgroup = min(1024, n)     # v rows per DMA group
    nvg = n // vgroup
    rpg = vgroup // 128       # row tiles per v group

    import contextlib
    with contextlib.ExitStack() as ctx:
        singles = ctx.enter_context(tc.tile_pool(name="singles", bufs=1))
        qk_stream = ctx.enter_context(tc.tile_pool(name="qk_stream", bufs=6))
        vstream = ctx.enter_context(tc.tile_pool(name="vstream", bufs=2 * nkt + 2))
        pt_pool = ctx.enter_context(tc.tile_pool(name="pt_pool", bufs=3))
        ostage = ctx.enter_context(tc.tile_pool(name="ostage", bufs=3))
        u_pool = ctx.enter_context(tc.tile_pool(name="u_pool", bufs=2))
        nrm_pool = ctx.enter_context(tc.tile_pool(name="nrm_pool", bufs=2))
        ps_scores = ctx.enter_context(
            tc.tile_pool(name="ps_scores", bufs=2, space="PSUM"))
        ps_acc = ctx.enter_context(
            tc.tile_pool(name="ps_acc", bufs=1, space="PSUM"))
        ps_small = ctx.enter_context(
            tc.tile_pool(name="ps_small", bufs=2, space="PSUM"))

        # ---- weights to SBUF ----
        wq_sb = singles.tile([128, nkt, 128], BF16, tag="wq")
        wk_sb = singles.tile([128, nkt, 128], BF16, tag="wk")
        wv_sb = singles.tile([128, nkt, 128], BF16, tag="wv")
        for kt in range(nkt):
            nc.sync.dma_start(out=wq_sb[:, kt, :], in_=wq[kt * 128:(kt + 1) * 128, :])
            nc.sync.dma_start(out=wk_sb[:, kt, :], in_=wk[kt * 128:(kt + 1) * 128, :])
            nc.sync.dma_start(out=wv_sb[:, kt, :], in_=wv[kt * 128:(kt + 1) * 128, :])
        wo0_sb = singles.tile([64, DOUT], BF16, tag="wo0")
        wo1_sb = singles.tile([64, DOUT], BF16, tag="wo1")
        nc.sync.dma_start(out=wo0_sb, in_=wo[0:64, :])
        nc.sync.dma_start(out=wo1_sb, in_=wo[64:128, :])
        # ones row at partition 64 (for the K=1 broadcast matmul)
        ones_sb = singles.tile([65, 64], F32, tag="ones")
        nc.vector.memset(ones_sb[64:65, :], 1.0)

        # ---- Q/K projections: QT/KT chunks [128, 512] (partition = head dim) ----
        qt_tiles, kt_tiles = [], []
        for i in range(nch):
            for src, w_sb, store, tagp in (
                (qT, wq_sb, qt_tiles, "qt"),
                (kT, wk_sb, kt_tiles, "kt"),
            ):
                ps = ps_small.tile([128, 512], F32, tag="w", name=f"ps_{tagp}{i}")
                for kt in range(nkt):
                    st = qk_stream.tile([128, 512], BF16, tag="qkst",
                                        name=f"st_{tagp}{i}_{kt}")
                    nc.sync.dma_start(
                        out=st,
                        in_=src[kt * 128:(kt + 1) * 128, i * 512:(i + 1) * 512])
                    nc.tensor.matmul(ps, lhsT=w_sb[:, kt, :], rhs=st,
                                     start=(kt == 0), stop=(kt == nkt - 1))
                dst = singles.tile([128, 512], BF16, tag=f"{tagp}{i}",
                                   name=f"{tagp}{i}")
                nc.vector.tensor_copy(dst, ps)
                store.append(dst)

        # ---- V projection: row-major tiles [128, 130] = V0 | 1 | V1 | 1 ----
        v_tiles = []
        for g in range(nvg):
            vts = []
            for kt in range(nkt):
                vt_in = vstream.tile([128, vgroup], BF16, tag="vst",
                                     name=f"vst{g}_{kt}")
                nc.sync.dma_start(
                    out=vt_in,
                    in_=vT[kt * 128:(kt + 1) * 128, g * vgroup:(g + 1) * vgroup])
                vts.append(vt_in)
            for r in range(rpg):
                jt = g * rpg + r
                pv = ps_small.tile([128, 128], F32, tag="w", name=f"pv{jt}")
                for kt in range(nkt):
                    nc.tensor.matmul(pv, lhsT=vts[kt][:, r * 128:(r + 1) * 128],
                                     rhs=wv_sb[:, kt, :],
                                     start=(kt == 0), stop=(kt == nkt - 1))
                v_t = singles.tile([128, 130], BF16, tag=f"v{jt}", name=f"v{jt}")
                nc.vector.tensor_copy(v_t[:, 0:64], pv[:, 0:64])
                nc.vector.tensor_copy(v_t[:, 65:129], pv[:, 64:128])
                nc.vector.memset(v_t[:, 64:65], 1.0)
                nc.vector.memset(v_t[:, 129:130], 1.0)
                v_tiles.append(v_t)

        # ---- attention + output projection ----
        for isup in range(nis):
            u = [
                u_pool.tile([64, is_chunk], BF16, tag="u0", name=f"u0_{isup}"),
                u_pool.tile([64, is_chunk], BF16, tag="u1", name=f"u1_{isup}"),
            ]
            for h in range(2):
                acc = ps_acc.tile([65, is_chunk], F32, tag="acc",
                                  name=f"acc{isup}_{h}")
                vlo = 0 if h == 0 else 65
                for jt in range(njt):
                    sc = ps_scores.tile([128, is_chunk], F32, tag="s",
                                        name=f"sc{isup}_{h}_{jt}")
                    ktile = kt_tiles[jt // 4]
                    for half in range(n_half):
                        ch = isup * n_half + half
                        nc.tensor.matmul(
                            sc[:, half * 512:(half + 1) * 512],
                            lhsT=ktile[h * 64:(h + 1) * 64,
                                       (jt % 4) * 128:(jt % 4) * 128 + 128],
                            rhs=qt_tiles[ch][h * 64:(h + 1) * 64, :],
                            start=True, stop=True)
                    pt = pt_pool.tile([128, is_chunk], BF16, tag="pt",
                                      name=f"pt{isup}_{h}_{jt}")
                    nc.scalar.activation(pt, sc, EXP, scale=0.125)
                    for half in range(n_half):
                        nc.tensor.matmul(
                            acc[:, half * 512:(half + 1) * 512],
                            lhsT=v_tiles[jt][:, vlo:vlo + 65],
                            rhs=pt[:, half * 512:(half + 1) * 512],
                            start=(jt == 0), stop=(jt == njt - 1))
                # normalize: u[h] = acc[0:64] / acc[64]
                for half in range(n_half):
                    sl = slice(half * 512, (half + 1) * 512)
                    rec = nrm_pool.tile([65, 512], F32, tag="rec",
                                        name=f"rec{isup}_{h}_{half}")
                    nc.vector.reciprocal(rec[64:65, :], acc[64:65, sl])
                    bc_ps = ps_small.tile([64, 512], F32, tag="w",
                                          name=f"bc_ps{isup}_{h}_{half}")
                    nc.tensor.matmul(bc_ps, lhsT=ones_sb[64:65, :],
                                     rhs=rec[64:65, :], start=True, stop=True)
                    bc_sb = nrm_pool.tile([64, 512], F32, tag="bc",
                                          name=f"bc_sb{isup}_{h}_{half}")
                    nc.vector.tensor_copy(bc_sb, bc_ps)
                    nc.vector.tensor_mul(u[h][:, sl], acc[0:64, sl], bc_sb)
            # output projection for this chunk of q rows
            for it in range(is_chunk // 128):
                row0 = isup * is_chunk + it * 128
                for wc in range(DOUT // 512):
                    po = ps_small.tile([128, 512], F32, tag="w",
                                       name=f"po{isup}_{it}_{wc}")
                    nc.tensor.matmul(po, lhsT=u[0][:, it * 128:(it + 1) * 128],
                                     rhs=wo0_sb[:, wc * 512:(wc + 1) * 512],
                                     start=True, stop=False)
                    nc.tensor.matmul(po, lhsT=u[1][:, it * 128:(it + 1) * 128],
                                     rhs=wo1_sb[:, wc * 512:(wc + 1) * 512],
                                     start=False, stop=True)
                    ot = ostage.tile([128, 512], F32, tag="ot",
                                     name=f"ot{isup}_{it}_{wc}")
                    nc.vector.tensor_copy(ot, po)
                    nc.sync.dma_start(
                        out=out[row0:row0 + 128, wc * 512:(wc + 1) * 512],
                        in_=ot)


def build(n=N, din=DIN):
    nc = bacc.Bacc("TRN2", target_bir_lowering=False, debug=False,
                   num_devices=NCORES)
    qT = nc.dram_tensor("qT", [din, n], BF16, kind="ExternalInput").ap()
    kT = nc.dram_tensor("kT", [din, n], BF16, kind="ExternalInput").ap()
    vT = nc.dram_tensor("vT", [din, n], BF16, kind="ExternalInput").ap()
    wq = nc.dram_tensor("wq", [din, DPC], BF16, kind="ExternalInput").ap()
    wk = nc.dram_tensor("wk", [din, DPC], BF16, kind="ExternalInput").ap()
    wv = nc.dram_tensor("wv", [din, DPC], BF16, kind="ExternalInput").ap()
    wo = nc.dram_tensor("wo", [DPC, DOUT], BF16, kind="ExternalInput").ap()
    out = nc.dram_tensor("out", [n, DOUT], F32, kind="ExternalOutput").ap()
    with tile.TileContext(nc) as tc:
        emit(tc, [out], [qT, kT, vT, wq, wk, wv, wo], n, din)
    nc.compile()
    return nc


_NC_CACHE = {}


def _get_nc(n=N, din=DIN):
    key = (n, din)
    if key not in _NC_CACHE:
        _NC_CACHE[key] = build(n, din)
    return _NC_CACHE[key]


def make_in_maps(q, k, v, W_q, W_k, W_v, W_o):
    bf = ml_dtypes.bfloat16
    qT = np.ascontiguousarray(np.asarray(q, dtype=np.float32).T).astype(bf)
    kT = np.ascontiguousarray(np.asarray(k, dtype=np.float32).T).astype(bf)
    vT = np.ascontiguousarray(np.asarray(v, dtype=np.float32).T).astype(bf)
    W_q = np.asarray(W_q, dtype=np.float32)
    W_k = np.asarray(W_k, dtype=np.float32)
    W_v = np.asarray(W_v, dtype=np.float32)
    W_o = np.asarray(W_o, dtype=np.float32)
    in_maps = []
    for c in range(NCORES):
        sl = slice(DPC * c, DPC * (c + 1))
        in_maps.append({
            "qT": qT, "kT": kT, "vT": vT,
            "wq": np.ascontiguousarray(W_q[:, sl]).astype(bf),
            "wk": np.ascontiguousarray(W_k[:, sl]).astype(bf),
            "wv": np.ascontiguousarray(W_v[:, sl]).astype(bf),
            "wo": np.ascontiguousarray(W_o[sl, :]).astype(bf),
        })
    return in_maps


def run(q, k, v, W_q, W_k, W_v, W_o, trace=False):
    n = q.shape[0]
    nc = _get_nc(n=n, din=q.shape[1])
    in_maps = make_in_maps(q, k, v, W_q, W_k, W_v, W_o)
    res = run_bass_kernel_spmd(nc, in_maps, list(range(NCORES)), trace=trace)
    out = res.results[0]["out"].astype(np.float32)
    for c in range(1, NCORES):
        out += res.results[c]["out"]
    return out, res


def kernel(q, k, v, W_q, W_k, W_v, W_o):
    out, _ = run(q, k, v, W_q, W_k, W_v, W_o)
    return out


# revision 6
# speedup vs baseline: 1.1542x; 1.1542x over previous
"""Multi-head attention (N=4096, D=1024, 16 heads) on 8 trn2 NeuronCores.

Sharding: tensor-parallel over heads. Each core owns 2 heads (128 of the
1024 QKV projection columns / W_o rows), computes its heads' attention
fully on-device, applies its W_o row-slice, and returns a partial
[4096, 1024] output. The host sums the 8 partials (the "all-reduce").

Device kernel per core (all matmuls bf16, accumulation fp32 in PSUM):
  1. QT/KT = (W_qc^T q^T), stored [128=2*64 head dims, 4096] in SBUF.
     V stored row-major [row-tile 128, d 0:64 | ones | d 64:128 | ones].
  2. Per head h, per 1024-row q chunk, streaming over 32 key tiles:
     scores^T tile = K_h^T-slice @ Q_h-chunk -> PSUM [128, 1024];
     P = exp(scores/8) via ScalarE direct PSUM->SBUF bf16;
     [u; r]^T += (V_h | 1)^T P^T accumulated in PSUM [65, 1024]
     (row 64 is the softmax denominator r).
  3. normalize: recip(r) on DVE, broadcast across 64 partitions with a
     K=1 matmul against ones, u_norm = u * bcast -> bf16.
  4. out-partial = u_norm^T @ W_o-slice (two K=64 matmuls accumulated),
     DMA PSUM -> DRAM fp32.
"""

import numpy as np
import ml_dtypes

import concourse.bass as bass
import concourse.tile as tile
from concourse import bacc, mybir
from concourse.bass_utils import run_bass_kernel_spmd

BF16 = mybir.dt.bfloat16
F32 = mybir.dt.float32
EXP = mybir.ActivationFunctionType.Exp

N = 4096
DIN = 1024
DOUT = 1024
NCORES = 8
DPC = 128  # dims per core = 2 heads * 64
HD = 64


def emit(tc, outs, ins, n, din):
    nc = tc.nc
    qT, kT, vT, wq, wk, wv, wo = ins
    out = outs[0]

    nkt = din // 128          # contraction tiles for projections
    nch = n // 512            # 512-wide column chunks of QT/KT
    njt = n // 128            # key row tiles
    is_chunk = min(1024, n)   # q rows processed per attention sweep
    nis = n // is_chunk
    n_half = is_chunk // 512
    vgroup = min(1024, n)     # v rows per DMA group
    nvg = n // vgroup
    rpg = vgroup // 128       # row tiles per v group

    import contextlib
    with contextlib.ExitStack() as ctx:
        singles = ctx.enter_context(tc.tile_pool(name="singles", bufs=1))
        qk_stream = ctx.enter_context(tc.tile_pool(name="qk_stream", bufs=6))
        vstream = ctx.enter_context(tc.tile_pool(name="vstream", bufs=2 * nkt + 2))
        pt_pool = ctx.enter_context(tc.tile_pool(name="pt_pool", bufs=3))
        ostage = ctx.enter_context(tc.tile_pool(name="ostage", bufs=3))
        u_pool = ctx.enter_context(tc.tile_pool(name="u_pool", bufs=2))
        nrm_pool = ctx.enter_context(tc.tile_pool(name="nrm_pool", bufs=2))
        ps_scores = ctx.enter_context(
            tc.tile_pool(name="ps_scores", bufs=2, space="PSUM"))
        ps_acc = ctx.enter_context(
            tc.tile_pool(name="ps_acc", bufs=1, space="PSUM"))
        ps_small = ctx.enter_context(
            tc.tile_pool(name="ps_small", bufs=2, space="PSUM"))

        # ---- weights to SBUF ----
        wq_sb = singles.tile([128, nkt, 128], BF16, tag="wq")
        wk_sb = singles.tile([128, nkt, 128], BF16, tag="wk")
        wv_sb = singles.tile([128, nkt, 128], BF16, tag="wv")
        for kt in range(nkt):
            nc.sync.dma_start(out=wq_sb[:, kt, :], in_=wq[kt * 128:(kt + 1) * 128, :])
            nc.sync.dma_start(out=wk_sb[:, kt, :], in_=wk[kt * 128:(kt + 1) * 128, :])
            nc.sync.dma_start(out=wv_sb[:, kt, :], in_=wv[kt * 128:(kt + 1) * 128, :])
        wo0_sb = singles.tile([64, DOUT], BF16, tag="wo0")
        wo1_sb = singles.tile([64, DOUT], BF16, tag="wo1")
        nc.sync.dma_start(out=wo0_sb, in_=wo[0:64, :])
        nc.sync.dma_start(out=wo1_sb, in_=wo[64:128, :])
        # ones row at partition 64 (for the K=1 broadcast matmul)
        ones_sb = singles.tile([65, 64], BF16, tag="ones")
        nc.vector.memset(ones_sb[64:65, :], 1.0)

        # ---- projection chunk emitters ----
        qt_tiles = [None] * nch
        kt_tiles = [None] * nch

        def qk_chunk(src, w_sb, store, tagp, i):
            """Project one 512-column chunk of QT/KT; yields micro-units."""
            ps = ps_small.tile([128, 512], F32, tag="w", name=f"ps_{tagp}{i}")
            for kt in range(nkt):
                def unit(kt=kt, ps=ps):
                    st = qk_stream.tile([128, 512], BF16, tag="qkst",
                                        name=f"st_{tagp}{i}_{kt}")
                    nc.sync.dma_start(
                        out=st,
                        in_=src[kt * 128:(kt + 1) * 128, i * 512:(i + 1) * 512])
                    nc.tensor.matmul(ps, lhsT=w_sb[:, kt, :], rhs=st,
                                     start=(kt == 0), stop=(kt == nkt - 1))
                yield unit
            def fin(ps=ps):
                dst = singles.tile([128, 512], BF16, tag=f"{tagp}{i}",
                                   name=f"{tagp}{i}")
                nc.vector.tensor_copy(dst, ps)
                store[i] = dst
            yield fin

        # V projection: row-major tiles [128, 130] = V0 | 1 | V1 | 1
        v_tiles = [None] * njt

        def v_group(g):
            vts = []
            def dmas():
                for kt in range(nkt):
                    vt_in = vstream.tile([128, vgroup], BF16, tag="vst",
                                         name=f"vst{g}_{kt}")
                    nc.sync.dma_start(
                        out=vt_in,
                        in_=vT[kt * 128:(kt + 1) * 128,
                               g * vgroup:(g + 1) * vgroup])
                    vts.append(vt_in)
            yield dmas
            for r in range(rpg):
                jt = g * rpg + r
                pv_box = []
                def mms(r=r, jt=jt, pv_box=pv_box):
                    pv = ps_small.tile([128, 128], F32, tag="w", name=f"pv{jt}")
                    pv_box.append(pv)
                    for kt in range(nkt):
                        nc.tensor.matmul(pv,
                                         lhsT=vts[kt][:, r * 128:(r + 1) * 128],
                                         rhs=wv_sb[:, kt, :],
                                         start=(kt == 0), stop=(kt == nkt - 1))
                yield mms
                def fin(jt=jt, pv_box=pv_box):
                    pv = pv_box[0]
                    v_t = singles.tile([128, 130], BF16, tag=f"v{jt}",
                                       name=f"v{jt}")
                    nc.vector.tensor_copy(v_t[:, 0:64], pv[:, 0:64])
                    nc.vector.tensor_copy(v_t[:, 65:129], pv[:, 64:128])
                    nc.vector.memset(v_t[:, 64:65], 1.0)
                    nc.vector.memset(v_t[:, 129:130], 1.0)
                    v_tiles[jt] = v_t
                yield fin

        # K fully, V fully, Q chunks 0..1 up front; Q 2.. dripped into the
        # attention loop.
        for i in range(nch):
            for u_ in qk_chunk(kT, wk_sb, kt_tiles, "kt", i):
                u_()
        for g in range(nvg):
            for u_ in v_group(g):
                u_()
        upfront_q = min(2, nch)
        for i in range(upfront_q):
            for u_ in qk_chunk(qT, wq_sb, qt_tiles, "qt", i):
                u_()
        drip = []
        for i in range(upfront_q, nch):
            drip.extend(qk_chunk(qT, wq_sb, qt_tiles, "qt", i))
        drip = list(reversed(drip))  # pop from the end

        # ---- software-pipelined attention ----
        # Per flat step t: [epilogue drip] scores(t) + exp(t) are emitted
        # before the V-matmuls of step t-1, so the in-order PE queue always
        # has the next scores ready for ScalarE before it blocks on exp(t-1)
        # consumers.
        def normalize(acc, u_h, tag):
            for half in range(n_half):
                sl = slice(half * 512, (half + 1) * 512)
                rec = nrm_pool.tile([65, 512], F32, tag="rec",
                                    name=f"rec{tag}_{half}")
                nc.vector.reciprocal(rec[64:65, :], acc[64:65, sl])
                recb = nrm_pool.tile([65, 512], BF16, tag="recb",
                                     name=f"recb{tag}_{half}")
                nc.vector.tensor_copy(recb[64:65, :], rec[64:65, :])
                bc_ps = ps_small.tile([64, 512], F32, tag="w",
                                      name=f"bc_ps{tag}_{half}")
                nc.tensor.matmul(bc_ps, lhsT=ones_sb[64:65, :],
                                 rhs=recb[64:65, :], start=True, stop=True)
                bc_sb = nrm_pool.tile([64, 512], F32, tag="bc",
                                      name=f"bc_sb{tag}_{half}")
                nc.vector.tensor_copy(bc_sb, bc_ps)
                nc.vector.tensor_mul(u_h[:, sl], acc[0:64, sl], bc_sb)

        def out_chunk(u, isup, it, wc):
            def unit():
                row0 = isup * is_chunk + it * 128
                po = ps_small.tile([128, 512], F32, tag="w",
                                   name=f"po{isup}_{it}_{wc}")
                nc.tensor.matmul(po, lhsT=u[0][:, it * 128:(it + 1) * 128],
                                 rhs=wo0_sb[:, wc * 512:(wc + 1) * 512],
                                 start=True, stop=False)
                nc.tensor.matmul(po, lhsT=u[1][:, it * 128:(it + 1) * 128],
                                 rhs=wo1_sb[:, wc * 512:(wc + 1) * 512],
                                 start=False, stop=True)
                ot = ostage.tile([128, 512], F32, tag="ot",
                                 name=f"ot{isup}_{it}_{wc}")
                nc.vector.tensor_copy(ot, po)
                nc.sync.dma_start(
                    out=out[row0:row0 + 128, wc * 512:(wc + 1) * 512],
                    in_=ot)
            return unit

        steps = [(isup, h, jt)
                 for isup in range(nis) for h in range(2) for jt in range(njt)]
        pending_v = None  # (acc, vslice, pt, jt) for step t-1
        pending_norm = None  # (acc, u_h, tag) once a head's jt loop is done
        epi = []  # out-projection units, dripped one per step
        u = None
        accs = {}

        for t, (isup, h, jt) in enumerate(steps):
            if jt == 0:
                if h == 0:
                    u = [
                        u_pool.tile([64, is_chunk], BF16, tag="u0",
                                    name=f"u0_{isup}"),
                        u_pool.tile([64, is_chunk], BF16, tag="u1",
                                    name=f"u1_{isup}"),
                    ]
                accs[(isup, h)] = ps_acc.tile([65, is_chunk], F32, tag="acc",
                                              name=f"acc{isup}_{h}")
            # drip: at most one projection micro-unit, then one epilogue unit
            if drip:
                drip.pop()()
            # scores + exp for step t
            sc = ps_scores.tile([128, is_chunk], F32, tag="s",
                                name=f"sc{isup}_{h}_{jt}")
            ktile = kt_tiles[jt // 4]
            for half in range(n_half):
                ch = isup * n_half + half
                nc.tensor.matmul(
                    sc[:, half * 512:(half + 1) * 512],
                    lhsT=ktile[h * 64:(h + 1) * 64,
                               (jt % 4) * 128:(jt % 4) * 128 + 128],
                    rhs=qt_tiles[ch][h * 64:(h + 1) * 64, :],
                    start=True, stop=True)
            pt = pt_pool.tile([128, is_chunk], BF16, tag="pt",
                              name=f"pt{isup}_{h}_{jt}")
            nc.scalar.activation(pt, sc, EXP, scale=0.125)
            # V matmuls for step t-1
            if pending_v is not None:
                acc_p, vslice_p, pt_p, last_p = pending_v
                for half in range(n_half):
                    nc.tensor.matmul(
                        acc_p[:, half * 512:(half + 1) * 512],
                        lhsT=vslice_p,
                        rhs=pt_p[:, half * 512:(half + 1) * 512],
                        start=(last_p == 0), stop=(last_p == njt - 1))
            acc = accs[(isup, h)]
            vlo = 0 if h == 0 else 65
            pending_v = (acc, v_tiles[jt][:, vlo:vlo + 65], pt, jt)
            # head seam: normalize previous head now that its V-matmuls are in
            if pending_norm is not None:
                normalize(*pending_norm)
                pending_norm = None
            elif epi:
                epi.pop()()
            if jt == njt - 1:
                pending_norm = (acc, u[h], f"{isup}_{h}")
                if h == 1:
                    assert not epi
                    epi = [out_chunk(u, isup, it, wc)
                           for it in range(is_chunk // 128)
                           for wc in range(DOUT // 512)]
                    epi.reverse()

        # tail: last head's V matmuls, its normalize, remaining epilogue
        acc_p, vslice_p, pt_p, last_p = pending_v
        for half in range(n_half):
            nc.tensor.matmul(
                acc_p[:, half * 512:(half + 1) * 512],
                lhsT=vslice_p,
                rhs=pt_p[:, half * 512:(half + 1) * 512],
                start=(last_p == 0), stop=(last_p == njt - 1))
        if pending_norm is not None:
            normalize(*pending_norm)
        while drip:
            drip.pop()()
        for unit in reversed(epi):
            unit()


def build(n=N, din=DIN):
    nc = bacc.Bacc("TRN2", target_bir_lowering=False, debug=False,
                   num_devices=NCORES)
    qT = nc.dram_tensor("qT", [din, n], BF16, kind="ExternalInput").ap()
    kT = nc.dram_tensor("kT", [din, n], BF16, kind="ExternalInput").ap()
    vT = nc.dram_tensor("vT", [din, n], BF16, kind="ExternalInput").ap()
    wq = nc.dram_tensor("wq", [din, DPC], BF16, kind="ExternalInput").ap()
    wk = nc.dram_tensor("wk", [din, DPC], BF16, kind="ExternalInput").ap()
    wv = nc.dram_tensor("wv", [din, DPC], BF16, kind="ExternalInput").ap()
    wo = nc.dram_tensor("wo", [DPC, DOUT], BF16, kind="ExternalInput").ap()
    out = nc.dram_tensor("out", [n, DOUT], F32, kind="ExternalOutput").ap()
    with tile.TileContext(nc) as tc:
        emit(tc, [out], [qT, kT, vT, wq, wk, wv, wo], n, din)
    nc.compile()
    return nc


_NC_CACHE = {}


def _get_nc(n=N, din=DIN):
    key = (n, din)
    if key not in _NC_CACHE:
        _NC_CACHE[key] = build(n, din)
    return _NC_CACHE[key]


def make_in_maps(q, k, v, W_q, W_k, W_v, W_o):
    bf = ml_dtypes.bfloat16
    qT = np.ascontiguousarray(np.asarray(q, dtype=np.float32).T).astype(bf)
    kT = np.ascontiguousarray(np.asarray(k, dtype=np.float32).T).astype(bf)
    vT = np.ascontiguousarray(np.asarray(v, dtype=np.float32).T).astype(bf)
    W_q = np.asarray(W_q, dtype=np.float32)
    W_k = np.asarray(W_k, dtype=np.float32)
    W_v = np.asarray(W_v, dtype=np.float32)
    W_o = np.asarray(W_o, dtype=np.float32)
    in_maps = []
    for c in range(NCORES):
        sl = slice(DPC * c, DPC * (c + 1))
        in_maps.append({
            "qT": qT, "kT": kT, "vT": vT,
            "wq": np.ascontiguousarray(W_q[:, sl]).astype(bf),
            "wk": np.ascontiguousarray(W_k[:, sl]).astype(bf),
            "wv": np.ascontiguousarray(W_v[:, sl]).astype(bf),
            "wo": np.ascontiguousarray(W_o[sl, :]).astype(bf),
        })
    return in_maps


def run(q, k, v, W_q, W_k, W_v, W_o, trace=False):
    n = q.shape[0]
    nc = _get_nc(n=n, din=q.shape[1])
    in_maps = make_in_maps(q, k, v, W_q, W_k, W_v, W_o)
    res = run_bass_kernel_spmd(nc, in_maps, list(range(NCORES)), trace=trace)
    out = res.results[0]["out"].astype(np.float32)
    for c in range(1, NCORES):
        out += res.results[c]["out"]
    return out, res


def kernel(q, k, v, W_q, W_k, W_v, W_o):
    out, _ = run(q, k, v, W_q, W_k, W_v, W_o)
    return out
